# revision 16
# baseline (speedup 1.0000x reference)
"""DeepSeekMoE forward on 8 TRN2 cores — v3.

Sharding: routed expert c -> core c; shared experts 8-way H-sliced;
router replicated. vs v2:

  - scatter/gather batched into single indirect DMAs (offset AP carries
    all 4096/1152 indices) — v2 spent ~130us of critical path on 41
    separate descgen ops spaced ~3us apart by sequencer latency.
  - router softplus/top-2 chain runs as ONE full-width island
    ([P,32,8] tiles) instead of 4 sequential 33-op islands: latency
    ~22us instead of ~91us; in-place buffer reuse keeps SBUF flat.
  - separate PSUM + staging pools for shared vs routed phases: v2's
    shared pool rotation serialized shared blocks 2/3 behind the
    routed halves (88us PE idle while waiting on the gather).
  - shared blocks shrunk to 512 tokens (8 blocks) for finer-grained
    gap filling and smaller xb/hTs footprints.
"""

import sys
from contextlib import ExitStack

if "/opt/trn_rl_repo" not in sys.path:
    sys.path.insert(0, "/opt/trn_rl_repo")

import numpy as np
import ml_dtypes

import concourse.bass as bass
import concourse.mybir as mybir
import concourse.tile as tile
from concourse import bacc
from concourse.bass import IndirectOffsetOnAxis
from concourse.bass_utils import run_bass_kernel_spmd

F32 = mybir.dt.float32
BF16 = mybir.dt.bfloat16
I32 = mybir.dt.int32
AF = mybir.ActivationFunctionType
OP = mybir.AluOpType
AX = mybir.AxisListType

N_CORES = 8
D = 1024
H = 4096
HS = 1024
E = 8
P = 128

NB = 512           # shared-expert token block
RXC = 128          # router moving-chunk (tokens)
CAP = 1152
HALVES = [(0, 5), (5, 4)]   # (start st-tile, n st-tiles) of cap/128=9


def _chunks(n, step=512):
    out, o = [], 0
    while o < n:
        out.append((o, min(step, n - o)))
        o += step
    return out


def build_nc(n_tok: int, cap: int, num_devices: int = N_CORES):
    assert n_tok % NB == 0 and cap % P == 0
    nc = bacc.Bacc("TRN2", target_bir_lowering=False, debug=False,
                   num_devices=num_devices)
    aps = {}

    def dram(name, shape, dt, kind="ExternalInput"):
        aps[name] = nc.dram_tensor(name, shape, dt, kind=kind).ap()

    TT = n_tok // P
    dram("xT", [D, n_tok], F32)          # router moving operand
    dram("xTb", [D, n_tok], BF16)        # shared-expert moving operand
    dram("xrowsb", [n_tok, D], BF16)     # gather source
    dram("rn", [n_tok, E], F32)
    dram("wrn", [D, 2 * E], F32)
    dram("brbn", [2 * E, 1], F32)
    dram("esel", [P, E], F32)
    dram("ones32", [1, P], F32)
    dram("onescol", [P, 1], F32)
    dram("triu128", [P, P], F32)         # [j,i]=1 if j<i
    dram("triu32", [TT, TT], F32)
    dram("id16", [16, 16], F32)
    dram("id128b", [P, P], BF16)
    dram("payt", [P, TT, 2], F32)        # [:,tt,0]=tt*128+p, [:,tt,1]=0
    dram("w1", [D, H], BF16)
    dram("b1", [P, H // P], F32)
    dram("w2", [H, D], BF16)
    dram("sw1", [D, HS], BF16)
    dram("sb1", [P, HS // P], F32)
    dram("sw2", [HS, D], BF16)
    dram("scat", [cap, 2], F32, kind="ExternalOutput")
    dram("out_sh", [n_tok, D], F32, kind="ExternalOutput")
    dram("out_rt", [cap, D], F32, kind="ExternalOutput")
    dram("cnt_t", [1, 1], F32, kind="ExternalOutput")

    with tile.TileContext(nc) as tc:
        with ExitStack() as es:
            _emit(es, tc, nc, aps, n_tok, cap)
    nc.compile()
    return nc


def _emit(es, tc, nc, aps, n_tok, cap):
    TT = n_tok // P
    DS = D // P
    NTC = cap // P

    A = type("A", (), aps)

    cpool = es.enter_context(tc.tile_pool(name="const", bufs=1))
    rxpool = es.enter_context(tc.tile_pool(name="rx", bufs=2))
    spool = es.enter_context(tc.tile_pool(name="rscratch", bufs=1))
    rpsum = es.enter_context(tc.tile_pool(name="rpsum", bufs=2, space="PSUM"))
    xpool = es.enter_context(tc.tile_pool(name="xb", bufs=2))
    xgpool = es.enter_context(tc.tile_pool(name="xgt", bufs=1))
    w1pool = es.enter_context(tc.tile_pool(name="w1b", bufs=2))
    swpool = es.enter_context(tc.tile_pool(name="swb", bufs=1))
    w2rpool = es.enter_context(tc.tile_pool(name="w2r", bufs=1))
    hspool = es.enter_context(tc.tile_pool(name="hTs", bufs=1))
    hrpool = es.enter_context(tc.tile_pool(name="hTr", bufs=1))
    yspool = es.enter_context(tc.tile_pool(name="ysgS", bufs=2))
    yrpool = es.enter_context(tc.tile_pool(name="ysgR", bufs=2))
    psumS = es.enter_context(tc.tile_pool(name="psumS", bufs=3, space="PSUM"))
    psumR = es.enter_context(tc.tile_pool(name="psumR", bufs=3, space="PSUM"))

    def ctile(shape, dt, name):
        return cpool.tile(shape, dt, name=name, tag=name)

    def stile(shape, name, dt=F32, bufs=None):
        return spool.tile(shape, dt, name=name, tag=name, bufs=bufs)

    def rps(shape, name, dt=F32):
        return rpsum.tile(shape, dt, name=name, tag="ps_r")

    def load_const(name, shape, dt):
        t = ctile(shape, dt, name + "_sb")
        nc.sync.dma_start(t[:], aps[name][:])
        return t

    # ---- constants ----
    wrn_sb = ctile([P, DS, 2 * E], F32, "wrn_sb")
    nc.sync.dma_start(wrn_sb[:], A.wrn.rearrange("(ds p) e -> p ds e", p=P))
    brbn_sb = load_const("brbn", [2 * E, 1], F32)
    esel_sb = load_const("esel", [P, E], F32)
    ones32_sb = load_const("ones32", [1, P], F32)
    onescol_sb = load_const("onescol", [P, 1], F32)
    triu128_sb = load_const("triu128", [P, P], F32)
    triu32_sb = load_const("triu32", [TT, TT], F32)
    id16_sb = load_const("id16", [16, 16], F32)
    id128b_sb = load_const("id128b", [P, P], BF16)
    b1_sb = load_const("b1", [P, H // P], F32)
    sb1_sb = load_const("sb1", [P, HS // P], F32)

    # ---- router matmuls: noisy = [logits | pre-softplus] -> lgnl ----
    lgnl = stile([P, TT, 2 * E], "lgnl")
    for rb in range(n_tok // RXC):
        xr = rxpool.tile([P, DS, RXC], F32, name="xr", tag="xr")
        nc.gpsimd.dma_start(
            xr[:],
            A.xT[:, rb * RXC:(rb + 1) * RXC].rearrange(
                "(ds p) t -> p ds t", p=P))
        ps_r = rpsum.tile([2 * E, RXC], F32, name="ps_r", tag="ps_r")
        for ds in range(DS):
            nc.tensor.matmul(ps_r[:], wrn_sb[:, ds, :], xr[:, ds, :],
                             start=(ds == 0), stop=(ds == DS - 1))
        lgch = stile([2 * E, RXC], "lgch", bufs=1)
        nc.vector.tensor_scalar(lgch[:], ps_r[:], brbn_sb[:], None,
                                op0=OP.add)
        for k in range(RXC // P):
            tps = psumR.tile([P, 2 * E], F32, name="tps_r", tag="ps")
            nc.tensor.transpose(tps[:], lgch[:, k * P:(k + 1) * P], id16_sb[:])
            tt = (rb * RXC) // P + k
            nc.vector.tensor_copy(lgnl[:, tt, :], tps[:])

    # ---- router chain, one full-width island; B0-B3 reused in place ----
    shp = [P, TT, E]
    lg = lgnl[:, :, 0:E]
    nl = lgnl[:, :, E:2 * E]
    B0 = stile(shp, "B0"); B1 = stile(shp, "B1")
    B2 = stile(shp, "B2"); B3 = stile(shp, "B3")
    rn_sb = stile(shp, "rn_sb")
    nc.gpsimd.dma_start(rn_sb[:],
                        A.rn.rearrange("(t p) e -> p t e", p=P))
    V, S = nc.vector, nc.scalar
    # compensated softplus: sp = s0 + (uu / exp_c(s0) - 1)
    S.activation(B0[:], nl, AF.Exp)                       # B0 = e0
    S.activation(B1[:], B0[:], AF.Ln)                     # B1 = l0
    V.tensor_tensor(B1[:], nl, B1[:], op=OP.subtract)     # B1 = r0
    V.tensor_tensor(B1[:], B0[:], B1[:], op=OP.mult)      # B1 = t0
    V.tensor_tensor(B1[:], B0[:], B1[:], op=OP.add)       # B1 = ee
    V.tensor_scalar_add(B1[:], B1[:], 1.0)                # B1 = uu
    S.activation(B0[:], B1[:], AF.Ln)                     # B0 = s0
    S.activation(B2[:], B0[:], AF.Exp)                    # B2 = e1
    S.activation(B3[:], B2[:], AF.Ln)                     # B3 = l1
    V.tensor_tensor(B3[:], B0[:], B3[:], op=OP.subtract)  # B3 = r1
    V.tensor_tensor(B3[:], B2[:], B3[:], op=OP.mult)      # B3 = t1
    V.tensor_tensor(B3[:], B2[:], B3[:], op=OP.add)       # B3 = e1p
    V.reciprocal(B2[:], B3[:])                            # B2 = re1
    V.tensor_tensor(B2[:], B1[:], B2[:], op=OP.mult)      # B2 = dd
    V.tensor_scalar_add(B2[:], B2[:], -1.0)               # B2 = dm
    V.tensor_tensor(B2[:], B0[:], B2[:], op=OP.add)       # B2 = sp
    V.tensor_tensor(B2[:], rn_sb[:], B2[:], op=OP.mult)   # B2 = noise
    V.tensor_tensor(B2[:], lg, B2[:], op=OP.add)          # B2 = noisy
    m1 = stile([P, TT], "m1")
    V.tensor_reduce(m1[:], B2[:], axis=AX.X, op=OP.max)
    m1b = m1[:, :, None].broadcast_to(shp)
    V.tensor_tensor(B0[:], B2[:], m1b, op=OP.is_equal)    # B0 = eq(top1)
    V.tensor_scalar_mul(B0[:], B0[:], 1e30)
    V.tensor_tensor(B0[:], B2[:], B0[:], op=OP.subtract)  # B0 = noisy2
    m2 = stile([P, TT], "m2")
    V.tensor_reduce(m2[:], B0[:], axis=AX.X, op=OP.max)
    m2b = m2[:, :, None].broadcast_to(shp)
    V.tensor_tensor(B0[:], B2[:], m2b, op=OP.is_ge)       # B0 = ge (top2 sel)
    V.tensor_tensor(B1[:], B2[:], m1b, op=OP.subtract)    # B1 = shd
    S.activation(B1[:], B1[:], AF.Exp)                    # B1 = ex
    V.tensor_tensor(B1[:], B1[:], B0[:], op=OP.mult)      # B1 = gg
    den = stile([P, TT], "den")
    V.tensor_reduce(den[:], B1[:], axis=AX.X, op=OP.add)
    rden = stile([P, TT], "rden")
    V.reciprocal(rden[:], den[:])
    V.tensor_tensor(B1[:], B1[:], rden[:, :, None].broadcast_to(shp),
                    op=OP.mult)                           # B1 = gate8
    eselb = esel_sb[:, None, :].broadcast_to(shp)
    V.tensor_tensor(B3[:], B1[:], eselb, op=OP.mult)
    gate = stile([P, TT], "gate")
    V.tensor_reduce(gate[:], B3[:], axis=AX.X, op=OP.add)
    V.tensor_tensor(B3[:], B0[:], eselb, op=OP.mult)
    mask = stile([P, TT], "mask")
    V.tensor_reduce(mask[:], B3[:], axis=AX.X, op=OP.add)

    # ---- compaction: slot = prefix(mask); unselected -> cap ----
    cntp = rps([TT, 1], "cntp")
    nc.tensor.matmul(cntp[:], mask[:], onescol_sb[:], start=True, stop=True)
    cnt_sb = stile([TT, 1], "cnt_sb")
    nc.scalar.activation(cnt_sb[:], cntp[:], AF.Copy)
    ecsp = rps([1, TT], "ecsp")
    nc.tensor.matmul(ecsp[:], cnt_sb[:], triu32_sb[:], start=True, stop=True)
    ecs_row = stile([1, TT], "ecs_row")
    nc.scalar.activation(ecs_row[:], ecsp[:], AF.Copy)
    totp = rps([1, 1], "totp")
    nc.tensor.matmul(totp[:], cnt_sb[:], onescol_sb[:TT, :], start=True, stop=True)
    tot_sb = stile([1, 1], "tot_sb")
    nc.scalar.activation(tot_sb[:], totp[:], AF.Copy)
    nc.sync.dma_start(A.cnt_t[:], tot_sb[:])

    posp = rps([P, TT], "posp")
    nc.tensor.matmul(posp[:], triu128_sb[:], mask[:], start=True, stop=False)
    nc.tensor.matmul(posp[:], ones32_sb[:1, :], ecs_row[:1, :],
                     start=False, stop=True)
    pos = stile([P, TT], "pos")
    nc.scalar.activation(pos[:], posp[:], AF.Copy)
    # pos_final = pos*mask + (1-mask)*cap, in place
    V.tensor_tensor(pos[:], pos[:], mask[:], op=OP.mult)
    pm_b = stile([P, TT], "pm_b")
    V.tensor_scalar(pm_b[:], mask[:], -float(cap), float(cap),
                    op0=OP.mult, op1=OP.add)              # cap*(1-mask)
    V.tensor_tensor(pos[:], pos[:], pm_b[:], op=OP.add)
    pos_i = stile([P, TT], "pos_i", I32)
    V.tensor_copy(pos_i[:], pos[:])

    # ---- slot tables: ONE batched scatter; OOB (pos=cap) dropped ----
    pay = stile([P, TT, 2], "pay")
    nc.sync.dma_start(pay[:], A.payt[:])
    V.tensor_copy(pay[:, :, 1], gate[:])
    for tt in range(TT):
        nc.gpsimd.indirect_dma_start(
            out=A.scat[:],
            out_offset=IndirectOffsetOnAxis(ap=pos_i[:, tt:tt + 1], axis=0),
            in_=pay[:, tt, :],
            in_offset=None,
            bounds_check=cap - 1,
            oob_is_err=False)
    scat_sb = stile([P, NTC, 2], "scat_sb")
    nc.gpsimd.dma_start(scat_sb[:],
                        A.scat.rearrange("(st p) c -> p st c", p=P))
    idx_i = stile([P, NTC], "idx_i", I32)
    V.tensor_copy(idx_i[:], scat_sb[:, :, 0])

    # ---- batched gather per half + transpose to xgT [d, slot] ----
    def emit_gather(h):
        st0, nst = HALVES[h]
        xga = xgpool.tile([P, nst, D], BF16, name=f"xga{h}", tag=f"xga{h}")
        for sl in range(nst):
            nc.gpsimd.indirect_dma_start(
                out=xga[:, sl, :], in_=A.xrowsb[:],
                in_offset=IndirectOffsetOnAxis(
                    ap=idx_i[:, st0 + sl:st0 + sl + 1], axis=0),
                out_offset=None,
                bounds_check=n_tok - 1,
                oob_is_err=False)
        xgT = xgpool.tile([P, DS, nst * P], BF16, name=f"xgT{h}",
                          tag=f"xgT{h}")
        for sl in range(nst):
            for dp in range(DS):
                tps = psumR.tile([P, P], BF16, name="tpsg", tag="ps")
                nc.tensor.transpose(tps[:], xga[:, sl, dp * P:(dp + 1) * P],
                                    id128b_sb[:])
                nc.scalar.activation(xgT[:, dp, sl * P:(sl + 1) * P], tps[:],
                                     AF.Copy)
        return xgT

    # ---- FFN building blocks (bf16 operands, fp32 psum) ----
    def gemm1(xsrc, nb, w1b_t, hT_t, bias_sb, bias_off, nsub, chunk, relu_eng,
              pp):
        ch = _chunks(nb, chunk)
        for hs in range(nsub):
            pss = [pp.tile([P, cw], F32, name="ps_g1", tag="ps")
                   for (_, cw) in ch]
            for ds in range(DS):
                for ci, (no, nw) in enumerate(ch):
                    nc.tensor.matmul(
                        pss[ci][:], w1b_t[:, ds, hs * P:(hs + 1) * P],
                        xsrc[:, ds, no:no + nw],
                        start=(ds == 0), stop=(ds == DS - 1))
            for ci, (no, nw) in enumerate(ch):
                bcol = bias_sb[:, bias_off + hs:bias_off + hs + 1]
                if relu_eng == "scalar":
                    nc.scalar.activation(
                        hT_t[:, hs, no:no + nw], pss[ci][:], AF.Relu,
                        bias=bcol)
                else:
                    nc.vector.tensor_scalar(
                        hT_t[:, hs, no:no + nw], pss[ci][:], bcol, 0.0,
                        op0=OP.add, op1=OP.max)

    # ---- routed FFN halves ----
    def emit_routed_half(h, xgT):
        st0, nst = HALVES[h]
        ntok_h = nst * P
        hT_h = hrpool.tile([P, H // P, ntok_h], BF16, name=f"hTr{h}",
                           tag="hTr")
        for hb in range(H // 512):
            w1b = w1pool.tile([P, DS, 512], BF16, name="w1b", tag="w1b")
            for (ho, hw) in _chunks(512, 256):
                nc.sync.dma_start(
                    w1b[:, :, ho:ho + hw],
                    A.w1[:, hb * 512 + ho:hb * 512 + ho + hw].rearrange(
                        "(ds p) h -> p ds h", p=P))
            gemm1(xgT, ntok_h, w1b, hT_h[:, hb * 4:(hb + 1) * 4, :],
                  b1_sb, hb * 4, 4, chunk=320 if ntok_h == 640 else 512,
                  relu_eng="scalar", pp=psumR)
        for dh in range(2):
            w2h = w2rpool.tile([P, H // P, 512], BF16, name="w2h", tag="w2h")
            for (ho, hw) in _chunks(H, 1024):
                nc.sync.dma_start(
                    w2h[:, ho // P:(ho + hw) // P, :],
                    A.w2[ho:ho + hw, dh * 512:(dh + 1) * 512].rearrange(
                        "(hs p) d -> p hs d", p=P))
            for tl in range(nst):
                ps = psumR.tile([P, 512], F32, name="ps_g2", tag="ps")
                for hsb in range(H // P):
                    nc.tensor.matmul(
                        ps[:], hT_h[:, hsb, tl * P:(tl + 1) * P],
                        w2h[:, hsb, :],
                        start=(hsb == 0), stop=(hsb == H // P - 1))
                tt = st0 + tl
                ystg = yrpool.tile([P, 512], F32, name="ystgR", tag="ystgR")
                nc.scalar.activation(ystg[:], ps[:], AF.Copy,
                                     scale=scat_sb[:, tt, 1:2])
                nc.sync.dma_start(
                    A.out_rt[tt * P:(tt + 1) * P, dh * 512:(dh + 1) * 512],
                    ystg[:])

    # ---- shared-expert weights: loaded once, reused by all blocks ----
    sw1b = swpool.tile([P, DS, HS], BF16, name="sw1b", tag="sw1b")
    for (ho_, hw_) in _chunks(HS, 256):
        nc.sync.dma_start(
            sw1b[:, :, ho_:ho_ + hw_],
            A.sw1[:, ho_:ho_ + hw_].rearrange("(ds p) h -> p ds h", p=P))
    sw2b = swpool.tile([P, HS // P, D], BF16, name="sw2b", tag="sw2b")
    for (do_, dw_) in _chunks(D, 256):
        nc.sync.dma_start(
            sw2b[:, :, do_:do_ + dw_],
            A.sw2[:, do_:do_ + dw_].rearrange("(hs p) d -> p hs d", p=P))

    # ---- shared-expert block (NB tokens) ----
    def shared_block(b):
        tok0 = b * NB
        NT = NB // P
        xb = xpool.tile([P, DS, NB], BF16, name="xb", tag="xb")
        for (no_, nw_) in (_chunks(NB, 256) if b == 0 else [(0, NB)]):
            nc.sync.dma_start(
                xb[:, :, no_:no_ + nw_],
                A.xTb[:, tok0 + no_:tok0 + no_ + nw_].rearrange(
                    "(ds p) t -> p ds t", p=P))
        hTs = hspool.tile([P, HS // P, NB], BF16, name="hTs", tag="hTs")
        gemm1(xb, NB, sw1b, hTs, sb1_sb, 0, HS // P, chunk=512,
              relu_eng="vector", pp=psumS)
        for tt in range(NT):
            pss = [psumS.tile([P, 512], F32, name="ps_s2", tag="ps")
                   for _ in range(2)]
            for hsb in range(HS // P):
                for ci in range(2):
                    nc.tensor.matmul(
                        pss[ci][:], hTs[:, hsb, tt * P:(tt + 1) * P],
                        sw2b[:, hsb, ci * 512:(ci + 1) * 512],
                        start=(hsb == 0), stop=(hsb == HS // P - 1))
            for ci in range(2):
                ystg = yspool.tile([P, 512], F32, name="ystgS", tag="ystgS")
                nc.scalar.activation(ystg[:], pss[ci][:], AF.Copy)
                nc.sync.dma_start(
                    A.out_sh[tok0 + tt * P:tok0 + (tt + 1) * P,
                             ci * 512:(ci + 1) * 512],
                    ystg[:])

    # ---- emission order: router machinery first (critical path); shared
    # blocks fill PE idle during chain/scatter; routed halves preempt ----
    xgT_A = emit_gather(0)
    xgT_B = emit_gather(1)
    for b in range(n_tok // NB):
        shared_block(b)
    with tc.high_priority():
        emit_routed_half(0, xgT_A)
        emit_routed_half(1, xgT_B)


# ---------------- host side ----------------

_NC_CACHE = {}


def _get_nc(n_tok, cap):
    key = (n_tok, cap)
    if key not in _NC_CACHE:
        _NC_CACHE[key] = build_nc(n_tok, cap)
    return _NC_CACHE[key]


def make_in_maps(n_tok, cap, x, router_noise, Wr, br, Wn, bn, rW1, rb1, rW2,
                 rb2, sW1, sb1, sW2, sb2):
    TT = n_tok // P
    BF = ml_dtypes.bfloat16
    xf = np.ascontiguousarray(x.reshape(n_tok, D))
    xT = np.ascontiguousarray(xf.T)
    xTb = xT.astype(BF)
    xrowsb = xf.astype(BF)
    rnf = np.ascontiguousarray(router_noise.reshape(n_tok, E)).astype(np.float32)
    wrn = np.ascontiguousarray(np.concatenate([Wr, Wn], axis=1)).astype(np.float32)
    brbn = np.concatenate([br, bn]).reshape(2 * E, 1).astype(np.float32)
    payt = np.zeros((P, TT, 2), np.float32)
    payt[:, :, 0] = (np.arange(TT)[None, :] * P + np.arange(P)[:, None])

    in_maps = []
    for c in range(N_CORES):
        se, hsl = c // 4, (c % 4) * HS
        esel = np.zeros((P, E), np.float32)
        esel[:, c] = 1.0
        in_maps.append({
            "xT": xT,
            "xTb": xTb,
            "xrowsb": xrowsb,
            "rn": rnf,
            "wrn": wrn,
            "brbn": brbn,
            "esel": esel,
            "ones32": np.ones((1, P), np.float32),
            "onescol": np.ones((P, 1), np.float32),
            "triu128": np.triu(np.ones((P, P), np.float32), 1),
            "triu32": np.triu(np.ones((TT, TT), np.float32), 1),
            "id16": np.eye(16, dtype=np.float32),
            "id128b": np.eye(P, dtype=BF),
            "payt": payt,
            "w1": np.ascontiguousarray(rW1[c]).astype(BF),
            "b1": np.ascontiguousarray(rb1[c].reshape(H // P, P).T),
            "w2": np.ascontiguousarray(rW2[c]).astype(BF),
            "sw1": np.ascontiguousarray(sW1[se][:, hsl:hsl + HS]).astype(BF),
            "sb1": np.ascontiguousarray(
                sb1[se][hsl:hsl + HS].reshape(HS // P, P).T),
            "sw2": np.ascontiguousarray(sW2[se][hsl:hsl + HS, :]).astype(BF),
        })
    return in_maps


def combine(x, results, n_tok, cap, rb2, sb2):
    acc = x.reshape(n_tok, D).astype(np.float32).copy()
    acc += sb2.sum(axis=0).astype(np.float32)
    for c in range(N_CORES):
        acc += results[c]["out_sh"]
    for c in range(N_CORES):
        n = int(round(float(results[c]["cnt_t"][0, 0])))
        assert n <= cap, f"core {c}: count {n} exceeds capacity {cap}"
        sc = results[c]["scat"]
        idx = np.rint(sc[:n, 0]).astype(np.int64)
        g = sc[:n, 1:2]
        acc[idx] += results[c]["out_rt"][:n] + g * rb2[c][None, :]
    return acc


def kernel(x, router_noise, topk, Wr, br, Wn, bn, rW1, rb1, rW2, rb2,
           sW1, sb1, sW2, sb2, _trace=False):
    assert int(topk) == 2
    x = np.asarray(x, np.float32)
    B, T, Dx = x.shape
    n_tok = B * T
    nc = _get_nc(n_tok, CAP)
    in_maps = make_in_maps(
        n_tok, CAP, x, np.asarray(router_noise, np.float32),
        np.asarray(Wr, np.float32), np.asarray(br, np.float32),
        np.asarray(Wn, np.float32), np.asarray(bn, np.float32),
        np.asarray(rW1, np.float32), np.asarray(rb1, np.float32),
        np.asarray(rW2, np.float32), np.asarray(rb2, np.float32),
        np.asarray(sW1, np.float32), np.asarray(sb1, np.float32),
        np.asarray(sW2, np.float32), np.asarray(sb2, np.float32))
    res = run_bass_kernel_spmd(nc, in_maps, core_ids=list(range(N_CORES)),
                               trace=_trace)
    out = combine(x, res.results, n_tok, CAP,
                  np.asarray(rb2, np.float32),
                  np.asarray(sb2, np.float32)).reshape(B, T, Dx)
    if _trace:
        return out, res
    return out
